# revision 1
# baseline (speedup 1.0000x reference)
"""Trainium2 Bass kernel for LpAlignEntropyLoss (B=2048, D=128, 2 views).

loss = mean_i ||z0_i - z1_i + eps||  -  0.5 * sum_v mean_i [ logsumexp_{j!=i}(-||zv_i - zv_j + eps||) - log(B-1) ]

Strategy (8 NeuronCores, batch-row sharded, 256 rows/core):
  dist^2[i,j] = n_i + n_j - 2 * z_i . z_j   (matmul trick, bf16 TensorE)
  - Each core gets z^T column-ROTATED so its own 256 rows are columns 0..255:
    the distance-matrix diagonal then sits at a compile-time-known position.
  - The diagonal is self-masked by accumulating -BIG*I into PSUM via a tiny
    identity matmul => exp(-sqrt(...)) underflows to exactly 0.
  - ScalarE pass 1: dist = Sqrt(-2*psum + n_row)   (bias = per-partition n_i)
  - ScalarE pass 2: Exp(-dist) with fused accum_out row-sum.
  - Align term: DVE diff+square of the first 256 columns, ones-matmul to
    reduce over D (partition axis).
  Host finishes the O(B) tail: log of the row-sums, sqrt of align rows, means.

eps=1e-8 is below fp32 ulp of every operand magnitude here; dropping it is
exact at fp32 resolution.
"""
import numpy as np
import ml_dtypes
from contextlib import ExitStack

B = 2048
D = 128
N_CORES = 8
R = B // N_CORES          # 256 rows per core
NCH = R // 128            # 2 row-chunks of 128 partitions
BIG = float(2 ** 20)
TAU = 1.0
LOG_NM1 = float(np.log(B - 1))

_cache: dict = {}


def _build():
    import concourse.tile as tile
    from concourse import bacc, mybir

    f32 = mybir.dt.float32
    bf16 = mybir.dt.bfloat16
    AF = mybir.ActivationFunctionType

    nc = bacc.Bacc("TRN2", target_bir_lowering=False, debug=False,
                   num_devices=N_CORES)

    zt_d = [nc.dram_tensor(f"zt{v}", [D, B], bf16, kind="ExternalInput").ap()
            for v in (0, 1)]
    nh_d = [nc.dram_tensor(f"nh{v}", [1, B], bf16, kind="ExternalInput").ap()
            for v in (0, 1)]
    nrow_d = nc.dram_tensor("nrow", [128, 2 * NCH], f32, kind="ExternalInput").ap()
    ident_d = nc.dram_tensor("ident", [128, 128], bf16, kind="ExternalInput").ap()
    ibig_d = nc.dram_tensor("ibig", [128, 128], bf16, kind="ExternalInput").ap()
    rowsums_d = nc.dram_tensor("rowsums", [2 * NCH, 128], f32,
                               kind="ExternalOutput").ap()
    alignsq_d = nc.dram_tensor("alignsq", [1, R], f32, kind="ExternalOutput").ap()

    with tile.TileContext(nc) as tc, ExitStack() as ctx:
        consts = ctx.enter_context(tc.tile_pool(name="consts", bufs=1))
        ztp = ctx.enter_context(tc.tile_pool(name="ztp", bufs=1))
        psum = ctx.enter_context(tc.tile_pool(name="psum", bufs=2, space="PSUM"))
        distp = ctx.enter_context(tc.tile_pool(name="distp", bufs=4))
        dumpp = ctx.enter_context(tc.tile_pool(name="dumpp", bufs=2))
        accp = ctx.enter_context(tc.tile_pool(name="accp", bufs=4))
        alnp = ctx.enter_context(tc.tile_pool(name="alnp", bufs=1))

        sb_zt = []
        for v in (0, 1):
            t_ = ztp.tile([D, B], bf16, tag=f"zt{v}")
            nc.sync.dma_start(t_[:], zt_d[v])
            sb_zt.append(t_)
        sb_nh = []
        for v in (0, 1):
            t_ = consts.tile([1, B], bf16, tag=f"nh{v}")
            nc.sync.dma_start(t_[:], nh_d[v])
            sb_nh.append(t_)
        sb_nrow = consts.tile([128, 2 * NCH], f32, tag="nrow")
        nc.sync.dma_start(sb_nrow[:], nrow_d)
        sb_ident = consts.tile([128, 128], bf16, tag="ident")
        nc.sync.dma_start(sb_ident[:], ident_d)
        sb_ibig = consts.tile([128, 128], bf16, tag="ibig")
        nc.sync.dma_start(sb_ibig[:], ibig_d)
        ones = consts.tile([128, 128], bf16, tag="ones")
        nc.vector.memset(ones[:], 1.0)

        # Phase A: matmuls + Sqrt passes (one ACT table set)
        dists = {}
        for v in (0, 1):
            for t in range(NCH):
                P = psum.tile([128, B], f32, tag="P")
                lhsT = sb_zt[v][:, t * 128:(t + 1) * 128]
                for s in range(4):
                    sl = slice(s * 512, (s + 1) * 512)
                    nc.tensor.matmul(P[:, sl], lhsT, sb_zt[v][:, sl],
                                     start=True, stop=False)
                    nc.tensor.matmul(P[:, sl], ones[0:1, :], sb_nh[v][0:1, sl],
                                     start=False, stop=(s != 0))
                dg = slice(t * 128, (t + 1) * 128)
                nc.tensor.matmul(P[:, dg], sb_ident[:], sb_ibig[:],
                                 start=False, stop=True)
                dist = distp.tile([128, B], f32, tag="dist")
                idx = v * NCH + t
                nc.scalar.activation(dist[:], P[:], AF.Sqrt,
                                     bias=sb_nrow[:, idx:idx + 1], scale=-2.0)
                dists[(v, t)] = dist

        # Phase B: Exp passes with fused row-sum (second ACT table set)
        for v in (0, 1):
            for t in range(NCH):
                dmp = dumpp.tile([128, B], bf16, tag="dump")
                acc = accp.tile([128, 1], f32, tag="acc")
                nc.scalar.activation(dmp[:], dists[(v, t)][:], AF.Exp,
                                     scale=-1.0 / TAU, accum_out=acc[:])
                idx = v * NCH + t
                nc.sync.dma_start(rowsums_d[idx:idx + 1, :], acc[:])

        # Align term: ||z0_i - z1_i||^2 for this core's 256 rows
        adiff = alnp.tile([128, R], bf16, tag="adiff")
        nc.vector.tensor_sub(adiff[:], sb_zt[0][:, :R], sb_zt[1][:, :R])
        asq = alnp.tile([128, R], bf16, tag="asq")
        nc.vector.tensor_mul(asq[:], adiff[:], adiff[:])
        aps = psum.tile([1, R], f32, tag="P")
        nc.tensor.matmul(aps[:], ones[:, 0:1], asq[:], start=True, stop=True)
        asb = alnp.tile([1, R], f32, tag="asb")
        nc.vector.tensor_copy(asb[:], aps[:])
        nc.sync.dma_start(alignsq_d[0:1, :], asb[:])

    nc.compile()
    return nc


def _prep_inputs(z0: np.ndarray, z1: np.ndarray):
    """Per-core input maps: rotate columns so core c's rows come first."""
    bf = ml_dtypes.bfloat16
    zs = [np.ascontiguousarray(z0, np.float32), np.ascontiguousarray(z1, np.float32)]
    norms = [(z.astype(np.float64) ** 2).sum(-1) for z in zs]  # [B] exact-ish
    eye = np.eye(128, dtype=np.float32)
    ident = eye.astype(bf)
    ibig = (-BIG * eye).astype(bf)
    in_maps = []
    for c in range(N_CORES):
        order = (np.arange(B) + c * R) % B
        m = {"ident": ident, "ibig": ibig}
        nrow = np.empty((128, 2 * NCH), np.float32)
        for v in (0, 1):
            zr = zs[v][order]                       # [B, D] rotated
            m[f"zt{v}"] = np.ascontiguousarray(zr.T).astype(bf)   # [D, B]
            m[f"nh{v}"] = (-0.5 * norms[v][order]).astype(np.float32)\
                .astype(bf).reshape(1, B)
            for t in range(NCH):
                nrow[:, v * NCH + t] = norms[v][order[t * 128:(t + 1) * 128]]\
                    .astype(np.float32)
        m["nrow"] = nrow
        in_maps.append(m)
    return in_maps


def kernel(z0: np.ndarray, z1: np.ndarray) -> np.ndarray:
    from concourse.bass_utils import run_bass_kernel_spmd

    if "nc" not in _cache:
        _cache["nc"] = _build()
    nc = _cache["nc"]

    in_maps = _prep_inputs(z0, z1)
    res = run_bass_kernel_spmd(nc, in_maps, core_ids=list(range(N_CORES)))

    rowsums = np.empty((2, B), np.float64)   # [view, global row]
    alignsq = np.empty((B,), np.float64)
    for c in range(N_CORES):
        out = res.results[c]
        rs = out["rowsums"].astype(np.float64)      # [2*NCH, 128]
        for v in (0, 1):
            for t in range(NCH):
                g0 = c * R + t * 128
                rowsums[v, g0:g0 + 128] = rs[v * NCH + t]
        alignsq[c * R:(c + 1) * R] = out["alignsq"][0].astype(np.float64)

    align_loss = np.sqrt(alignsq).mean()
    lme = np.log(rowsums) - LOG_NM1             # [2, B]
    entropy_loss = lme.mean()
    return np.float32(align_loss - entropy_loss)



# revision 3
# speedup vs baseline: 1.3351x; 1.3351x over previous
"""Trainium2 Bass kernel for LpAlignEntropyLoss (B=2048, D=128, 2 views).

loss = mean_i ||z0_i - z1_i + eps||  -  0.5 * sum_v mean_i [ logsumexp_{j!=i}(-||zv_i - zv_j + eps||) - log(B-1) ]

Strategy (8 NeuronCores, batch-row sharded, 256 rows/core, symmetric-half):
  dist^2[i,j] = n_i + n_j - 2 * z_i . z_j   (matmul trick, bf16 TensorE)
  The 16x16 grid of 128x128 blocks is covered once using symmetry:
  row-chunk p computes column blocks at ring distance 0..7 (one 1024-wide
  PSUM slab) plus its distance-8 block (shared 512-wide "extra" slab, all
  4 chunk/view quarters).  exp(-dist) row sums for distances 1..7 are also
  column-summed (stationary-operand matmuls: out[m,0] = sum_k E[k,m], PE
  cost ~ 0) and shipped to the transposed rows on the host, so every row
  still sees all 2047 partners while the ACT engine (the bottleneck:
  0.833 ns/elem for both Sqrt and Exp, dtype-independent) only processes
  ~56% of the full distance matrix.
  - diagonal self-masked by accumulating -BIG*I into PSUM (identity matmul)
  - ScalarE pass 1: dist = Sqrt(-2*psum + n_row)  (bias = per-partition n_i)
  - ScalarE pass 2: E = Exp(-dist); all Exp passes take their scale from an
    SBUF operand written after the last Sqrt -> the tile scheduler cannot
    interleave Sqrt/Exp, so exactly 2 activation-table loads are paid.
  - row sums on DVE (tensor_reduce), align term via DVE square + stationary
    matmuls.  One [128,38] f32 export tile -> one output DMA.
  Host finishes the O(B) tail: assemble rowsums, log, sqrt, means.

eps=1e-8 is below fp32 ulp of every operand magnitude here; dropping it is
exact at fp32 resolution.
"""
import numpy as np
import ml_dtypes
from contextlib import ExitStack

B = 2048
D = 128
N_CORES = 8
R = B // N_CORES          # 256 rows per core
W = 1280                  # local columns held per core (10 chunks)
MAIN = 1024               # main slab width (ring distance 0..7)
BIG = float(2 ** 20)
TAU = 1.0
LOG_NM1 = float(np.log(B - 1))

_cache: dict = {}


def _build():
    import concourse.tile as tile
    from concourse import bacc, mybir

    f32 = mybir.dt.float32
    bf16 = mybir.dt.bfloat16
    AF = mybir.ActivationFunctionType

    nc = bacc.Bacc("TRN2", target_bir_lowering=False, debug=False,
                   num_devices=N_CORES)

    zt_d = [nc.dram_tensor(f"zt{v}", [D, W], bf16, kind="ExternalInput").ap()
            for v in (0, 1)]
    nh_d = [nc.dram_tensor(f"nh{v}", [1, W], bf16, kind="ExternalInput").ap()
            for v in (0, 1)]
    nrow_d = nc.dram_tensor("nrow", [128, 4], f32, kind="ExternalInput").ap()
    nrowt_d = nc.dram_tensor("nrowt", [1, 512], bf16, kind="ExternalInput").ap()
    ident_d = nc.dram_tensor("ident", [128, 128], bf16, kind="ExternalInput").ap()
    ibig_d = nc.dram_tensor("ibig", [128, 128], bf16, kind="ExternalInput").ap()
    out_d = nc.dram_tensor("out", [128, 38], f32, kind="ExternalOutput").ap()

    with tile.TileContext(nc) as tc, ExitStack() as ctx:
        consts = ctx.enter_context(tc.tile_pool(name="consts", bufs=1))
        ztp = ctx.enter_context(tc.tile_pool(name="ztp", bufs=1))
        psum = ctx.enter_context(tc.tile_pool(name="psum", bufs=1, space="PSUM"))
        distp = ctx.enter_context(tc.tile_pool(name="distp", bufs=1))
        ep = ctx.enter_context(tc.tile_pool(name="ep", bufs=1))
        outp = ctx.enter_context(tc.tile_pool(name="outp", bufs=1))

        # ---- input DMAs, split across the two HWDGE queues (SP + ACT) ----
        sb_zt = []
        for v in (0, 1):
            t_ = ztp.tile([D, W], bf16, tag=f"zt{v}", name=f"sb_zt{v}")
            sb_zt.append(t_)
        # own rows + extra-slab rhs cols first, bulk later (SP queue)
        for v in (0, 1):
            nc.sync.dma_start(sb_zt[v][:, 0:256], zt_d[v][:, 0:256])
            nc.sync.dma_start(sb_zt[v][:, 1024:1280], zt_d[v][:, 1024:1280])
        for v in (0, 1):
            nc.sync.dma_start(sb_zt[v][:, 256:1024], zt_d[v][:, 256:1024])

        sb_nrow = consts.tile([128, 4], f32, tag="nrow", name="sb_nrow")
        nc.scalar.dma_start(sb_nrow[:], nrow_d)
        sb_ident = consts.tile([128, 128], bf16, tag="ident", name="sb_ident")
        nc.scalar.dma_start(sb_ident[:], ident_d)
        sb_ibig = consts.tile([128, 128], bf16, tag="ibig", name="sb_ibig")
        nc.scalar.dma_start(sb_ibig[:], ibig_d)
        sb_nrowt = consts.tile([1, 512], bf16, tag="nrowt", name="sb_nrowt")
        nc.scalar.dma_start(sb_nrowt[:], nrowt_d)
        sb_nh = []
        for v in (0, 1):
            t_ = consts.tile([1, W], bf16, tag=f"nh{v}", name=f"sb_nh{v}")
            sb_nh.append(t_)
        for v in (0, 1):
            nc.scalar.dma_start(sb_nh[v][0:1, 1024:1280], nh_d[v][0:1, 1024:1280])
        for v in (0, 1):
            nc.scalar.dma_start(sb_nh[v][0:1, 0:1024], nh_d[v][0:1, 0:1024])

        ones = consts.tile([128, 128], bf16, tag="ones", name="ones")
        nc.vector.memset(ones[:], 1.0)

        # ---- extra slab: the four distance-8 blocks, one per (view,chunk) ----
        pex = psum.tile([128, 4, 128], f32, tag="pex", name="pex")
        for q in range(4):
            v, t = q // 2, q % 2
            lhsT = sb_zt[v][:, t * 128:(t + 1) * 128]
            rhs = sb_zt[v][:, 1024 + t * 128:1024 + (t + 1) * 128]
            nc.tensor.matmul(pex[:, q, :], lhsT, rhs, start=True, stop=False)
            nc.tensor.matmul(pex[:, q, :], ones[0:1, :],
                             sb_nh[v][0:1, 1024 + t * 128:1024 + (t + 1) * 128],
                             start=False, stop=False)
            nc.tensor.matmul(pex[:, q, :], sb_nrowt[0:1, q * 128:(q + 1) * 128],
                             ones[0:1, 0:128], start=False, stop=True)

        dist_ex = distp.tile([128, 4, 128], f32, tag="dist_ex", name="dist_ex")
        nc.scalar.activation(dist_ex[:], pex[:], AF.Sqrt, bias=0.0, scale=-2.0)

        # ---- main slabs: ring distance 0..7 per (view, chunk) ----
        dists = {}
        for v in (0, 1):
            for t in range(2):
                P = psum.tile([128, MAIN], f32, tag="slab", bufs=3, name="P")
                w0 = t * 128
                lhsT = sb_zt[v][:, t * 128:(t + 1) * 128]
                for s in range(2):
                    sl = slice(s * 512, (s + 1) * 512)
                    wsl = slice(w0 + s * 512, w0 + (s + 1) * 512)
                    nc.tensor.matmul(P[:, sl], lhsT, sb_zt[v][:, wsl],
                                     start=True, stop=False)
                    nc.tensor.matmul(P[:, sl], ones[0:1, :], sb_nh[v][0:1, wsl],
                                     start=False, stop=(s != 0))
                nc.tensor.matmul(P[:, 0:128], sb_ident[:], sb_ibig[:],
                                 start=False, stop=True)
                idx = v * 2 + t
                dist = distp.tile([128, MAIN], f32, tag=f"dist{idx}",
                                  name=f"dist{idx}")
                nc.scalar.activation(dist[:], P[:], AF.Sqrt,
                                     bias=sb_nrow[:, idx:idx + 1], scale=-2.0)
                dists[idx] = dist

        # ---- align term: ||z0_i - z1_i||^2 for this core's 256 rows ----
        outP = psum.tile([128, 30], f32, tag="out", name="outP")
        adiff = outp.tile([128, 256], bf16, tag="adiff", name="adiff")
        nc.vector.tensor_sub(adiff[:], sb_zt[0][:, 0:256], sb_zt[1][:, 0:256])
        asq = outp.tile([128, 256], bf16, tag="asq", name="asq")
        nc.vector.tensor_mul(asq[:], adiff[:], adiff[:])
        for h in range(2):
            nc.tensor.matmul(outP[:, 28 + h:29 + h],
                             asq[:, h * 128:(h + 1) * 128], ones[:, 0:1],
                             start=True, stop=True)

        # ---- serialize all Exp after the last Sqrt: scale comes from an SBUF
        # operand derived from the last dist tile (2 ACT table loads total)
        negones = outp.tile([128, 1], f32, tag="negones", name="negones")
        import concourse.mybir as mb
        nc.vector.tensor_scalar(negones[:], dists[3][:, 0:1], 0.0, -1.0,
                                mb.AluOpType.mult, mb.AluOpType.add)

        export = outp.tile([128, 38], f32, tag="export", name="export")

        # ---- Exp passes + row sums (DVE) + column sums (stationary matmuls)
        e_ex = ep.tile([128, 4, 128], bf16, tag="e_ex", name="e_ex")
        nc.scalar.activation(e_ex[:], dist_ex[:], AF.Exp, scale=negones[:, 0:1])
        nc.vector.tensor_reduce(export[:, 34:38], e_ex[:],
                                mb.AxisListType.X, mb.AluOpType.add)

        for idx in range(4):
            E = ep.tile([128, MAIN], bf16, tag=f"e{idx}", name=f"e{idx}")
            nc.scalar.activation(E[:], dists[idx][:], AF.Exp,
                                 scale=negones[:, 0:1])
            nc.vector.tensor_reduce(export[:, 30 + idx:31 + idx], E[:],
                                    mb.AxisListType.X, mb.AluOpType.add)
            for b in range(1, 8):
                col = idx * 7 + (b - 1)
                nc.tensor.matmul(outP[:, col:col + 1],
                                 E[:, b * 128:(b + 1) * 128], ones[:, 0:1],
                                 start=True, stop=True)

        nc.vector.tensor_copy(export[:, 0:30], outP[:])
        nc.sync.dma_start(out_d, export[:])

    nc.compile()
    return nc


def _prep_inputs(z0: np.ndarray, z1: np.ndarray):
    """Per-core input maps: rotate columns so core c's rows come first."""
    bf = ml_dtypes.bfloat16
    zs = [np.ascontiguousarray(z0, np.float32), np.ascontiguousarray(z1, np.float32)]
    norms = [(z.astype(np.float64) ** 2).sum(-1) for z in zs]  # [B]
    eye = np.eye(128, dtype=np.float32)
    ident = eye.astype(bf)
    ibig = (-BIG * eye).astype(bf)
    in_maps = []
    for c in range(N_CORES):
        order = (np.arange(W) + c * R) % B
        m = {"ident": ident, "ibig": ibig}
        nrow = np.empty((128, 4), np.float32)
        nrowt = np.empty((1, 512), np.float32)
        for v in (0, 1):
            zr = zs[v][order]                                   # [W, D] rotated
            m[f"zt{v}"] = np.ascontiguousarray(zr.T).astype(bf)  # [D, W]
            m[f"nh{v}"] = (-0.5 * norms[v][order]).astype(np.float32)\
                .astype(bf).reshape(1, W)
            for t in range(2):
                n_i = norms[v][order[t * 128:(t + 1) * 128]].astype(np.float32)
                nrow[:, v * 2 + t] = n_i
                # extra slab adds n_i pre-scale (x -2 later): store -0.5*n_i
                nrowt[0, (v * 2 + t) * 128:(v * 2 + t + 1) * 128] = -0.5 * n_i
        m["nrow"] = nrow
        m["nrowt"] = nrowt.astype(bf)
        in_maps.append(m)
    return in_maps


def kernel(z0: np.ndarray, z1: np.ndarray) -> np.ndarray:
    from concourse.bass_utils import run_bass_kernel_spmd

    if "nc" not in _cache:
        _cache["nc"] = _build()
    nc = _cache["nc"]

    in_maps = _prep_inputs(z0, z1)
    res = run_bass_kernel_spmd(nc, in_maps, core_ids=list(range(N_CORES)))

    rowsums = np.zeros((2, B), np.float64)   # [view, global row]
    alignsq = np.empty((B,), np.float64)
    for c in range(N_CORES):
        out = res.results[c]["out"].astype(np.float64)   # [128, 38]
        for v in (0, 1):
            for t in range(2):
                idx = v * 2 + t
                own = ((2 * c + t) % 16) * 128
                # main (distances 0..7) + extra (distance 8) row sums
                rowsums[v, own:own + 128] += out[:, 30 + idx] + out[:, 34 + idx]
                # received column sums (distances 1..7, transposed rows)
                for b in range(1, 8):
                    g = ((2 * c + t + b) % 16) * 128
                    rowsums[v, g:g + 128] += out[:, idx * 7 + (b - 1)]
        alignsq[c * R:c * R + 128] = out[:, 28]
        alignsq[c * R + 128:c * R + 256] = out[:, 29]

    align_loss = np.sqrt(alignsq).mean()
    lme = np.log(rowsums) - LOG_NM1             # [2, B]
    entropy_loss = lme.mean()
    return np.float32(align_loss - entropy_loss)


# revision 7
# speedup vs baseline: 1.6368x; 1.2260x over previous
"""Trainium2 Bass kernel for LpAlignEntropyLoss (B=2048, D=128, 2 views).

loss = mean_i ||z0_i - z1_i + eps||  -  0.5 * sum_v mean_i [ logsumexp_{j!=i}(-||zv_i - zv_j + eps||) - log(B-1) ]

Strategy (8 NeuronCores, batch-row sharded, 256 rows/core, symmetric-half):
  dist^2[i,j] = n_i + n_j - 2 * z_i . z_j   (matmul trick, bf16 TensorE)
  The 16x16 grid of 128x128 blocks is covered once using symmetry:
  row-chunk p computes column blocks at ring distance 0..7 (one 1024-wide
  PSUM slab) plus its distance-8 block (shared 512-wide "extra" slab).
  exp(-dist) row sums for distances 1..7 are also column-summed
  (stationary-operand matmuls, PE cost ~ free size = 1) and shipped to the
  transposed rows on the host, so every row sees all 2047 partners while
  the ACT engine (the bottleneck: 0.833 ns/elem for Sqrt and Exp,
  dtype-independent) only processes ~56% of the full distance matrix.

  Norm terms are pure matmul tricks against sq = zt*zt (DVE):
   - n_j along the free axis:  lhsT = negh (all -0.5), rhs = sq  ->
     out[m,n] = -0.5 * n_j for every partition m.
   - n_i along partitions (extra slab): lhsT = sq chunk, rhs = negh.
   - n_i bias for main slabs: free-size-1 colnorm matmuls -> f32 bias.
  So only 4 input DMAs total (zt0 split in two + zt1 + ident/ibig consts);
  HWDGE descriptor-gen is ~625 ns and globally serialized, so few DMAs win.

  - diagonal self-masked by accumulating -BIG*I into PSUM (identity matmul)
  - ScalarE pass 1: dist = Sqrt(-2*psum + n_row)  (bias = per-partition n_i)
  - ScalarE pass 2: E = Exp(-dist); all Exp passes take their scale from an
    SBUF operand written after every Sqrt output -> the tile scheduler
    cannot interleave Sqrt/Exp, so exactly 2 activation-table loads are
    paid (and both get hoisted into the idle head).
  - row sums on DVE (tensor_reduce), align term via DVE square + stationary
    matmuls.  One [128,38] f32 export tile -> one output DMA.
  - warm-up matmuls (ones x ones into the extra-slab PSUM, later
    overwritten) keep PE continuously busy from t~0.4us so it reaches the
    full 2.4 GHz pstate before the real slab matmuls arrive.
  Host finishes the O(B) tail: assemble rowsums, log, sqrt, means.

eps=1e-8 is below fp32 ulp of every operand magnitude here; dropping it is
exact at fp32 resolution.
"""
import numpy as np
import ml_dtypes
from contextlib import ExitStack

B = 2048
D = 128
N_CORES = 8
R = B // N_CORES          # 256 rows per core
W = 1280                  # local columns held per core (10 chunks)
MAIN = 1024               # main slab width (ring distance 0..7)
BIG = float(2 ** 20)
TAU = 1.0
LOG_NM1 = float(np.log(B - 1))
N_WARMUP = 26             # PE pstate warm-up matmuls

_cache: dict = {}


def _build():
    import concourse.tile as tile
    from concourse import bacc, mybir
    import concourse.mybir as mb

    f32 = mybir.dt.float32
    bf16 = mybir.dt.bfloat16
    AF = mybir.ActivationFunctionType

    nc = bacc.Bacc("TRN2", target_bir_lowering=False, debug=False,
                   num_devices=N_CORES)

    zt_d = [nc.dram_tensor(f"zt{v}", [D, W], bf16, kind="ExternalInput").ap()
            for v in (0, 1)]
    consts_d = nc.dram_tensor("consts", [128, 256], bf16,
                              kind="ExternalInput").ap()
    out_d = nc.dram_tensor("out", [128, 38], f32, kind="ExternalOutput").ap()

    with tile.TileContext(nc) as tc, ExitStack() as ctx:
        consts = ctx.enter_context(tc.tile_pool(name="consts", bufs=1))
        ztp = ctx.enter_context(tc.tile_pool(name="ztp", bufs=1))
        psum = ctx.enter_context(tc.tile_pool(name="psum", bufs=1, space="PSUM"))
        distp = ctx.enter_context(tc.tile_pool(name="distp", bufs=1))
        ep = ctx.enter_context(tc.tile_pool(name="ep", bufs=1))
        outp = ctx.enter_context(tc.tile_pool(name="outp", bufs=1))

        # ---- input DMAs on the SP HWDGE queue ----
        sb_zt = []
        for v in (0, 1):
            t_ = ztp.tile([D, W], bf16, tag=f"zt{v}", name=f"sb_zt{v}")
            sb_zt.append(t_)
        nc.sync.dma_start(sb_zt[0][:, 0:640], zt_d[0][:, 0:640])
        nc.sync.dma_start(sb_zt[0][:, 640:1280], zt_d[0][:, 640:1280])
        nc.sync.dma_start(sb_zt[1][:], zt_d[1])
        sb_c = consts.tile([128, 256], bf16, tag="consts", name="sb_c")
        nc.sync.dma_start(sb_c[:], consts_d)
        ident = sb_c[:, 0:128]
        ibig = sb_c[:, 128:256]

        ones = consts.tile([128, 128], bf16, tag="ones", name="ones")
        nc.vector.memset(ones[:], 1.0)
        negh = consts.tile([128, 128], bf16, tag="negh", name="negh")
        nc.vector.memset(negh[:], -0.5)

        # PSUM layout: 3 rotating 2-bank slabs + 1-bank extra + 1-bank outP
        outP = psum.tile([128, 34], f32, tag="out", name="outP")
        pex = psum.tile([128, 4, 128], f32, tag="pex", name="pex")

        # PE pstate warm-up: harmless matmuls into pex (overwritten later)
        for _ in range(N_WARMUP):
            nc.tensor.matmul(pex[:, 0, :], ones[:], ones[:],
                             start=True, stop=True)

        # ---- sq = zt*zt (DVE); n_i colnorms for the f32 Sqrt bias ----
        sq = []
        for v in (0, 1):
            s_ = outp.tile([128, W], bf16, tag=f"sq{v}", name=f"sq{v}")
            sq.append(s_)
        nc.vector.tensor_mul(sq[0][:, 0:640], sb_zt[0][:, 0:640],
                             sb_zt[0][:, 0:640])
        nc.vector.tensor_mul(sq[0][:, 640:1280], sb_zt[0][:, 640:1280],
                             sb_zt[0][:, 640:1280])
        nc.vector.tensor_mul(sq[1][:], sb_zt[1][:], sb_zt[1][:])

        nrow = outp.tile([128, 4], f32, tag="nrow", name="nrow")
        for v in (0, 1):
            for t in range(2):
                nc.tensor.matmul(outP[:, 30 + 2 * v + t:31 + 2 * v + t],
                                 sq[v][:, t * 128:(t + 1) * 128], ones[:, 0:1],
                                 start=True, stop=True)
            nc.vector.tensor_copy(nrow[:, 2 * v:2 * v + 2],
                                  outP[:, 30 + 2 * v:32 + 2 * v])

        # ---- main slabs: ring distance 0..7 per (view, chunk) ----
        dists = {}
        for v in (0, 1):
            for t in range(2):
                P = psum.tile([128, MAIN], f32, tag="slab", bufs=3, name="P")
                w0 = t * 128
                lhsT = sb_zt[v][:, t * 128:(t + 1) * 128]
                for s in range(2):
                    sl = slice(s * 512, (s + 1) * 512)
                    wsl = slice(w0 + s * 512, w0 + (s + 1) * 512)
                    nc.tensor.matmul(P[:, sl], lhsT, sb_zt[v][:, wsl],
                                     start=True, stop=False)
                    nc.tensor.matmul(P[:, sl], negh[:], sq[v][:, wsl],
                                     start=False, stop=(s != 0))
                nc.tensor.matmul(P[:, 0:128], ident, ibig,
                                 start=False, stop=True)
                idx = v * 2 + t
                dist = distp.tile([128, MAIN], f32, tag=f"dist{idx}",
                                  name=f"dist{idx}")
                nc.scalar.activation(dist[:], P[:], AF.Sqrt,
                                     bias=nrow[:, idx:idx + 1], scale=-2.0)
                dists[idx] = dist

        # ---- extra slab: the four distance-8 blocks, one per (view,chunk) ----
        for q in range(4):
            v, t = q // 2, q % 2
            csl = slice(1024 + t * 128, 1024 + (t + 1) * 128)
            lhsT = sb_zt[v][:, t * 128:(t + 1) * 128]
            nc.tensor.matmul(pex[:, q, :], lhsT, sb_zt[v][:, csl],
                             start=True, stop=False)
            nc.tensor.matmul(pex[:, q, :], negh[:], sq[v][:, csl],
                             start=False, stop=False)
            nc.tensor.matmul(pex[:, q, :], sq[v][:, t * 128:(t + 1) * 128],
                             negh[:, 0:128], start=False, stop=True)
        dist_ex = distp.tile([128, 4, 128], f32, tag="dist_ex", name="dist_ex")
        nc.scalar.activation(dist_ex[:], pex[:], AF.Sqrt, bias=0.0, scale=-2.0)

        # ---- align term: ||z0_i - z1_i||^2 for this core's 256 rows ----
        adiff = outp.tile([128, 256], bf16, tag="adiff", name="adiff")
        nc.vector.tensor_sub(adiff[:], sb_zt[0][:, 0:256], sb_zt[1][:, 0:256])
        asq = outp.tile([128, 256], bf16, tag="asq", name="asq")
        nc.vector.tensor_mul(asq[:], adiff[:], adiff[:])
        for h in range(2):
            nc.tensor.matmul(outP[:, 28 + h:29 + h],
                             asq[:, h * 128:(h + 1) * 128], ones[:, 0:1],
                             start=True, stop=True)

        # ---- serialize all Exp after every Sqrt: Exp's scale operand is
        # derived from all 5 dist tiles -> exactly 2 ACT table loads.
        neg = outp.tile([128, 1], f32, tag="neg", name="neg")
        nc.vector.tensor_scalar(neg[:], dists[3][:, 0:1], 0.0, -1.0,
                                mb.AluOpType.mult, mb.AluOpType.add)
        for dep in (dists[0], dists[1], dists[2]):
            nc.vector.tensor_tensor(neg[:], neg[:], dep[:, 0:1],
                                    mb.AluOpType.bypass)
        nc.vector.tensor_tensor(neg[:], neg[:], dist_ex[:, 0, 0:1],
                                mb.AluOpType.bypass)

        export = outp.tile([128, 38], f32, tag="export", name="export")

        # ---- Exp passes + row sums (DVE) + column sums (stationary matmuls)
        for idx in range(4):
            E = ep.tile([128, MAIN], bf16, tag=f"e{idx}", name=f"e{idx}")
            nc.scalar.activation(E[:], dists[idx][:], AF.Exp,
                                 scale=neg[:, 0:1])
            nc.vector.tensor_reduce(export[:, 30 + idx:31 + idx], E[:],
                                    mb.AxisListType.X, mb.AluOpType.add)
            for b in range(1, 8):
                col = idx * 7 + (b - 1)
                nc.tensor.matmul(outP[:, col:col + 1],
                                 E[:, b * 128:(b + 1) * 128], ones[:, 0:1],
                                 start=True, stop=True)
        e_ex = ep.tile([128, 4, 128], bf16, tag="e_ex", name="e_ex")
        nc.scalar.activation(e_ex[:], dist_ex[:], AF.Exp, scale=neg[:, 0:1])
        nc.vector.tensor_reduce(export[:, 34:38], e_ex[:],
                                mb.AxisListType.X, mb.AluOpType.add)

        nc.vector.tensor_copy(export[:, 0:30], outP[:, 0:30])
        nc.sync.dma_start(out_d, export[:])

    nc.compile()
    return nc


def _prep_inputs(z0: np.ndarray, z1: np.ndarray):
    """Per-core input maps: rotate columns so core c's rows come first."""
    bf = ml_dtypes.bfloat16
    zs = [np.ascontiguousarray(z0, np.float32), np.ascontiguousarray(z1, np.float32)]
    eye = np.eye(128, dtype=np.float32)
    consts = np.concatenate([eye, -BIG * eye], axis=1).astype(bf)  # [128, 256]
    in_maps = []
    for c in range(N_CORES):
        order = (np.arange(W) + c * R) % B
        m = {"consts": consts}
        for v in (0, 1):
            zr = zs[v][order]                                    # [W, D] rotated
            m[f"zt{v}"] = np.ascontiguousarray(zr.T).astype(bf)  # [D, W]
        in_maps.append(m)
    return in_maps


def kernel(z0: np.ndarray, z1: np.ndarray) -> np.ndarray:
    from concourse.bass_utils import run_bass_kernel_spmd

    if "nc" not in _cache:
        _cache["nc"] = _build()
    nc = _cache["nc"]

    in_maps = _prep_inputs(z0, z1)
    res = run_bass_kernel_spmd(nc, in_maps, core_ids=list(range(N_CORES)))

    rowsums = np.zeros((2, B), np.float64)   # [view, global row]
    alignsq = np.empty((B,), np.float64)
    for c in range(N_CORES):
        out = res.results[c]["out"].astype(np.float64)   # [128, 38]
        for v in (0, 1):
            for t in range(2):
                idx = v * 2 + t
                own = ((2 * c + t) % 16) * 128
                # main (distances 0..7) + extra (distance 8) row sums
                rowsums[v, own:own + 128] += out[:, 30 + idx] + out[:, 34 + idx]
                # received column sums (distances 1..7, transposed rows)
                for b in range(1, 8):
                    g = ((2 * c + t + b) % 16) * 128
                    rowsums[v, g:g + 128] += out[:, idx * 7 + (b - 1)]
        alignsq[c * R:c * R + 128] = out[:, 28]
        alignsq[c * R + 128:c * R + 256] = out[:, 29]

    align_loss = np.sqrt(alignsq).mean()
    lme = np.log(rowsums) - LOG_NM1             # [2, B]
    entropy_loss = lme.mean()
    return np.float32(align_loss - entropy_loss)


# revision 11
# speedup vs baseline: 1.7890x; 1.0930x over previous
"""Trainium2 Bass kernel for LpAlignEntropyLoss (B=2048, D=128, 2 views).

loss = mean_i ||z0_i - z1_i + eps||  -  0.5 * sum_v mean_i [ logsumexp_{j!=i}(-||zv_i - zv_j + eps||) - log(B-1) ]

Strategy (8 NeuronCores, batch-row sharded, 256 rows/core, symmetric-half):
  dist^2[i,j] = n_i + n_j - 2 * z_i . z_j   (matmul trick, bf16 TensorE)
  The 16x16 grid of 128x128 blocks is covered once using symmetry:
  row-chunk p computes column blocks at ring distance 0..7 (one 1024-wide
  PSUM slab) plus its distance-8 block (shared 512-wide "extra" slab).
  exp(-dist) row sums for distances 1..7 are also column-summed
  (stationary-operand matmuls, PE cost ~ free size = 1) and shipped to the
  transposed rows on the host, so every row sees all 2047 partners while
  the ACT engine (the bottleneck: 0.833 ns/elem for Sqrt and Exp,
  dtype-independent) only processes ~56% of the full distance matrix.

  Norm terms are pure matmul tricks against sq = zt*zt (DVE):
   - n_j along the free axis:  lhsT = negh (all -0.5), rhs = sq  ->
     out[m,n] = -0.5 * n_j for every partition m.
   - n_i along partitions (extra slab): lhsT = sq chunk, rhs = negh.
   - n_i bias for main slabs: free-size-1 colnorm matmuls -> f32 bias.
  So only 4 input DMAs total (zt0 split in two + zt1 + ident/ibig consts);
  HWDGE descriptor-gen is ~625 ns and globally serialized, so few DMAs win.

  - diagonal self-masked by accumulating -BIG*I into PSUM (identity matmul)
  - ScalarE pass 1: dist = Sqrt(-2*psum + n_row)  (bias = per-partition n_i)
  - ScalarE pass 2: E = Exp(-dist); all Exp passes take their scale from an
    SBUF operand written after every Sqrt output -> the tile scheduler
    cannot interleave Sqrt/Exp, so exactly 2 activation-table loads are
    paid (and both get hoisted into the idle head).
  - row sums on DVE (tensor_reduce), align term via DVE square + stationary
    matmuls.  One [128,38] f32 export tile -> one output DMA.
  - warm-up matmuls (ones x ones into the extra-slab PSUM, later
    overwritten) keep PE continuously busy from t~0.4us so it reaches the
    full 2.4 GHz pstate before the real slab matmuls arrive.
  Host finishes the O(B) tail: assemble rowsums, log, sqrt, means.

eps=1e-8 is below fp32 ulp of every operand magnitude here; dropping it is
exact at fp32 resolution.
"""
import numpy as np
import ml_dtypes
from contextlib import ExitStack

B = 2048
D = 128
N_CORES = 8
R = B // N_CORES          # 256 rows per core
W = 1280                  # local columns held per core (10 chunks)
MAIN = 1024               # main slab width (ring distance 0..7)
BIG = float(2 ** 20)
TAU = 1.0
LOG_NM1 = float(np.log(B - 1))
N_WARMUP = 21             # PE pstate warm-up matmuls

_cache: dict = {}


def _build():
    import concourse.tile as tile
    from concourse import bacc, mybir
    import concourse.mybir as mb

    f32 = mybir.dt.float32
    bf16 = mybir.dt.bfloat16
    AF = mybir.ActivationFunctionType

    nc = bacc.Bacc("TRN2", target_bir_lowering=False, debug=False,
                   num_devices=N_CORES)

    zt_d = [nc.dram_tensor(f"zt{v}", [D, W], bf16, kind="ExternalInput").ap()
            for v in (0, 1)]
    consts_d = nc.dram_tensor("consts", [128, 256], bf16,
                              kind="ExternalInput").ap()
    out_d = nc.dram_tensor("out", [128, 38], f32, kind="ExternalOutput").ap()

    with tile.TileContext(nc) as tc, ExitStack() as ctx:
        consts = ctx.enter_context(tc.tile_pool(name="consts", bufs=1))
        ztp = ctx.enter_context(tc.tile_pool(name="ztp", bufs=1))
        psum = ctx.enter_context(tc.tile_pool(name="psum", bufs=1, space="PSUM"))
        distp = ctx.enter_context(tc.tile_pool(name="distp", bufs=1))
        ep = ctx.enter_context(tc.tile_pool(name="ep", bufs=1))
        outp = ctx.enter_context(tc.tile_pool(name="outp", bufs=1))

        # ---- input DMAs on the SP HWDGE queue ----
        sb_zt = []
        for v in (0, 1):
            t_ = ztp.tile([D, W], bf16, tag=f"zt{v}", name=f"sb_zt{v}")
            sb_zt.append(t_)
        sb_c = consts.tile([128, 256], bf16, tag="consts", name="sb_c")
        nc.sync.dma_start(sb_zt[0][:, 0:640], zt_d[0][:, 0:640])
        nc.sync.dma_start(sb_c[:], consts_d)
        nc.sync.dma_start(sb_zt[0][:, 640:1280], zt_d[0][:, 640:1280])
        nc.sync.dma_start(sb_zt[1][:], zt_d[1])
        ident = sb_c[:, 0:128]
        ibig = sb_c[:, 128:256]

        ones = consts.tile([128, 128], bf16, tag="ones", name="ones")
        nc.vector.memset(ones[:], 1.0)
        negh = consts.tile([128, 128], bf16, tag="negh", name="negh")
        nc.vector.memset(negh[:], -0.5)

        # PSUM layout: 3 rotating 2-bank slabs + 1-bank extra + 1-bank outP
        outP = psum.tile([128, 34], f32, tag="out", name="outP")
        pex = psum.tile([128, 4, 128], f32, tag="pex", name="pex")

        # PE pstate warm-up: harmless matmuls into pex (overwritten later)
        for _ in range(N_WARMUP):
            nc.tensor.matmul(pex[:, 0, :], ones[:], ones[:],
                             start=True, stop=True)

        # dummy early Sqrt on a const: the sqrt-table load attaches to this
        # instruction's (trivial) waits and runs in the idle head instead of
        # gating the first real Sqrt.
        dummy = outp.tile([128, 1], f32, tag="dummy", name="dummy")
        nc.scalar.activation(dummy[:], ones[:, 0:1], AF.Sqrt,
                             bias=0.0, scale=1.0)

        # ---- sq = zt*zt (DVE); n_i colnorms for the f32 Sqrt bias ----
        sq = []
        for v in (0, 1):
            s_ = outp.tile([128, W], bf16, tag=f"sq{v}", name=f"sq{v}")
            sq.append(s_)
        nc.vector.tensor_mul(sq[0][:, 0:640], sb_zt[0][:, 0:640],
                             sb_zt[0][:, 0:640])
        nc.vector.tensor_mul(sq[0][:, 640:1280], sb_zt[0][:, 640:1280],
                             sb_zt[0][:, 640:1280])
        nc.vector.tensor_mul(sq[1][:], sb_zt[1][:], sb_zt[1][:])

        nrow = outp.tile([128, 4], f32, tag="nrow", name="nrow")
        for v in (0, 1):
            for t in range(2):
                nc.tensor.matmul(outP[:, 30 + 2 * v + t:31 + 2 * v + t],
                                 sq[v][:, t * 128:(t + 1) * 128], ones[:, 0:1],
                                 start=True, stop=True)
            nc.vector.tensor_copy(nrow[:, 2 * v:2 * v + 2],
                                  outP[:, 30 + 2 * v:32 + 2 * v])

        # ---- main slabs: ring distance 0..7 per (view, chunk) ----
        dists = {}
        for v in (0, 1):
            for t in range(2):
                P = psum.tile([128, MAIN], f32, tag="slab", bufs=3, name="P")
                w0 = t * 128
                lhsT = sb_zt[v][:, t * 128:(t + 1) * 128]
                for s in range(2):
                    sl = slice(s * 512, (s + 1) * 512)
                    wsl = slice(w0 + s * 512, w0 + (s + 1) * 512)
                    nc.tensor.matmul(P[:, sl], lhsT, sb_zt[v][:, wsl],
                                     start=True, stop=False)
                    nc.tensor.matmul(P[:, sl], negh[:], sq[v][:, wsl],
                                     start=False, stop=(s != 0))
                nc.tensor.matmul(P[:, 0:128], ident, ibig,
                                 start=False, stop=True)
                idx = v * 2 + t
                dist = distp.tile([128, MAIN], f32, tag=f"dist{idx}",
                                  name=f"dist{idx}")
                nc.scalar.activation(dist[:], P[:], AF.Sqrt,
                                     bias=nrow[:, idx:idx + 1], scale=-2.0)
                dists[idx] = dist

        # ---- extra slab: the four distance-8 blocks, one per (view,chunk) ----
        for q in range(4):
            v, t = q // 2, q % 2
            csl = slice(1024 + t * 128, 1024 + (t + 1) * 128)
            lhsT = sb_zt[v][:, t * 128:(t + 1) * 128]
            nc.tensor.matmul(pex[:, q, :], lhsT, sb_zt[v][:, csl],
                             start=True, stop=False)
            nc.tensor.matmul(pex[:, q, :], negh[:], sq[v][:, csl],
                             start=False, stop=False)
            nc.tensor.matmul(pex[:, q, :], sq[v][:, t * 128:(t + 1) * 128],
                             negh[:, 0:128], start=False, stop=True)
        dist_ex = distp.tile([128, 4, 128], f32, tag="dist_ex", name="dist_ex")
        nc.scalar.activation(dist_ex[:], pex[:], AF.Sqrt, bias=0.0, scale=-2.0)

        # ---- align term: ||z0_i - z1_i||^2 for this core's 256 rows ----
        adiff = outp.tile([128, 256], bf16, tag="adiff", name="adiff")
        nc.vector.tensor_sub(adiff[:], sb_zt[0][:, 0:256], sb_zt[1][:, 0:256])
        asq = outp.tile([128, 256], bf16, tag="asq", name="asq")
        nc.vector.tensor_mul(asq[:], adiff[:], adiff[:])
        for h in range(2):
            nc.tensor.matmul(outP[:, 28 + h:29 + h],
                             asq[:, h * 128:(h + 1) * 128], ones[:, 0:1],
                             start=True, stop=True)

        # ---- serialize all Exp after every Sqrt: Exp's scale operand is
        # derived from all 5 dist tiles -> exactly 2 ACT table loads.
        neg = outp.tile([128, 1], f32, tag="neg", name="neg")
        nc.vector.tensor_scalar(neg[:], dists[0][:, 0:1], 0.0, -1.0,
                                mb.AluOpType.mult, mb.AluOpType.add)
        for dep in (dists[1], dists[2], dists[3]):
            nc.vector.tensor_tensor(neg[:], neg[:], dep[:, 0:1],
                                    mb.AluOpType.bypass)
        # (no dist_ex link: by the time dist3 exists, Sx's inputs are long
        # ready, so the scheduler's priority order keeps Sx before any Exp)

        export = outp.tile([128, 38], f32, tag="export", name="export")

        # ---- Exp passes + row sums + column sums (stationary matmuls) ----
        # e_ex first (its DVE reduce then overlaps the main Exp chain);
        # the LAST Exp uses ACT accum_out so no DVE reduce lands in the tail.
        e_ex = ep.tile([128, 4, 128], bf16, tag="e_ex", name="e_ex")
        nc.scalar.activation(e_ex[:], dist_ex[:], AF.Exp, scale=neg[:, 0:1])
        nc.vector.tensor_reduce(export[:, 34:38], e_ex[:],
                                mb.AxisListType.X, mb.AluOpType.add)
        for idx in range(4):
            E = ep.tile([128, MAIN], bf16, tag=f"e{idx}", name=f"e{idx}")
            if idx == 3:
                nc.scalar.activation(E[:], dists[idx][:], AF.Exp,
                                     scale=neg[:, 0:1],
                                     accum_out=export[:, 33:34])
            else:
                nc.scalar.activation(E[:], dists[idx][:], AF.Exp,
                                     scale=neg[:, 0:1])
                nc.vector.tensor_reduce(export[:, 30 + idx:31 + idx], E[:],
                                        mb.AxisListType.X, mb.AluOpType.add)
            for b in range(1, 8):
                col = idx * 7 + (b - 1)
                nc.tensor.matmul(outP[:, col:col + 1],
                                 E[:, b * 128:(b + 1) * 128], ones[:, 0:1],
                                 start=True, stop=True)

        nc.vector.tensor_copy(export[:, 0:30], outP[:, 0:30])
        nc.sync.dma_start(out_d, export[:])

    nc.compile()
    return nc


def _prep_inputs(z0: np.ndarray, z1: np.ndarray):
    """Per-core input maps: rotate columns so core c's rows come first."""
    bf = ml_dtypes.bfloat16
    zs = [np.ascontiguousarray(z0, np.float32), np.ascontiguousarray(z1, np.float32)]
    eye = np.eye(128, dtype=np.float32)
    consts = np.concatenate([eye, -BIG * eye], axis=1).astype(bf)  # [128, 256]
    in_maps = []
    for c in range(N_CORES):
        order = (np.arange(W) + c * R) % B
        m = {"consts": consts}
        for v in (0, 1):
            zr = zs[v][order]                                    # [W, D] rotated
            m[f"zt{v}"] = np.ascontiguousarray(zr.T).astype(bf)  # [D, W]
        in_maps.append(m)
    return in_maps


def kernel(z0: np.ndarray, z1: np.ndarray) -> np.ndarray:
    from concourse.bass_utils import run_bass_kernel_spmd

    if "nc" not in _cache:
        _cache["nc"] = _build()
    nc = _cache["nc"]

    in_maps = _prep_inputs(z0, z1)
    res = run_bass_kernel_spmd(nc, in_maps, core_ids=list(range(N_CORES)))

    rowsums = np.zeros((2, B), np.float64)   # [view, global row]
    alignsq = np.empty((B,), np.float64)
    for c in range(N_CORES):
        out = res.results[c]["out"].astype(np.float64)   # [128, 38]
        for v in (0, 1):
            for t in range(2):
                idx = v * 2 + t
                own = ((2 * c + t) % 16) * 128
                # main (distances 0..7) + extra (distance 8) row sums
                rowsums[v, own:own + 128] += out[:, 30 + idx] + out[:, 34 + idx]
                # received column sums (distances 1..7, transposed rows)
                for b in range(1, 8):
                    g = ((2 * c + t + b) % 16) * 128
                    rowsums[v, g:g + 128] += out[:, idx * 7 + (b - 1)]
        alignsq[c * R:c * R + 128] = out[:, 28]
        alignsq[c * R + 128:c * R + 256] = out[:, 29]

    align_loss = np.sqrt(alignsq).mean()
    lme = np.log(rowsums) - LOG_NM1             # [2, B]
    entropy_loss = lme.mean()
    return np.float32(align_loss - entropy_loss)


# revision 18
# speedup vs baseline: 1.7953x; 1.0035x over previous
"""Trainium2 Bass kernel for LpAlignEntropyLoss (B=2048, D=128, 2 views).

loss = mean_i ||z0_i - z1_i + eps||  -  0.5 * sum_v mean_i [ logsumexp_{j!=i}(-||zv_i - zv_j + eps||) - log(B-1) ]

Strategy (8 NeuronCores, batch-row sharded, 256 rows/core, symmetric-half):
  dist^2[i,j] = n_i + n_j - 2 * z_i . z_j   (matmul trick, bf16 TensorE)
  The 16x16 grid of 128x128 blocks is covered once using symmetry:
  row-chunk p computes column blocks at ring distance 0..7 (one 1024-wide
  PSUM slab) plus its distance-8 block (shared 512-wide "extra" slab).
  exp(-dist) row sums for distances 1..7 are also column-summed
  (stationary-operand matmuls, PE cost ~ free size = 1) and shipped to the
  transposed rows on the host, so every row sees all 2047 partners while
  the ACT engine (the bottleneck: 0.833 ns/elem for Sqrt and Exp,
  dtype-independent) only processes ~56% of the full distance matrix.

  Norm terms are pure matmul tricks against sq = zt*zt (DVE):
   - n_j along the free axis:  lhsT = negh (all -0.5), rhs = sq  ->
     out[m,n] = -0.5 * n_j for every partition m.
   - n_i along partitions (extra slab): lhsT = sq chunk, rhs = negh.
   - n_i bias for main slabs: free-size-1 colnorm matmuls -> f32 bias.
  So only 4 input DMAs total (zt0 split in two + zt1 + ident/ibig consts);
  HWDGE descriptor-gen is ~625 ns and globally serialized, so few DMAs win.

  - diagonal self-masked by accumulating -BIG*I into PSUM (identity matmul)
  - ScalarE pass 1: dist = Sqrt(-2*psum + n_row)  (bias = per-partition n_i)
  - ScalarE pass 2: E = Exp(-dist); all Exp passes take their scale from an
    SBUF operand written after every Sqrt output -> the tile scheduler
    cannot interleave Sqrt/Exp, so exactly 2 activation-table loads are
    paid (and both get hoisted into the idle head).
  - row sums on DVE (tensor_reduce), align term via DVE square + stationary
    matmuls.  One [128,38] f32 export tile -> one output DMA.
  - warm-up matmuls (ones x ones into the extra-slab PSUM, later
    overwritten) keep PE continuously busy from t~0.4us so it reaches the
    full 2.4 GHz pstate before the real slab matmuls arrive.
  Host finishes the O(B) tail: assemble rowsums, log, sqrt, means.

eps=1e-8 is below fp32 ulp of every operand magnitude here; dropping it is
exact at fp32 resolution.
"""
import numpy as np
import ml_dtypes
from contextlib import ExitStack

B = 2048
D = 128
N_CORES = 8
R = B // N_CORES          # 256 rows per core
W = 1280                  # local columns held per core (10 chunks)
MAIN = 1024               # main slab width (ring distance 0..7)
BIG = float(2 ** 20)
TAU = 1.0
LOG_NM1 = float(np.log(B - 1))
N_WARMUP = 24             # PE pstate warm-up matmuls

_cache: dict = {}


def _build():
    import concourse.tile as tile
    from concourse import bacc, mybir
    import concourse.mybir as mb

    f32 = mybir.dt.float32
    bf16 = mybir.dt.bfloat16
    AF = mybir.ActivationFunctionType

    nc = bacc.Bacc("TRN2", target_bir_lowering=False, debug=False,
                   num_devices=N_CORES)

    zt_d = [nc.dram_tensor(f"zt{v}", [D, W], bf16, kind="ExternalInput").ap()
            for v in (0, 1)]
    consts_d = nc.dram_tensor("consts", [128, 256], bf16,
                              kind="ExternalInput").ap()
    out_d = nc.dram_tensor("out", [128, 38], f32, kind="ExternalOutput").ap()

    with tile.TileContext(nc) as tc, ExitStack() as ctx:
        consts = ctx.enter_context(tc.tile_pool(name="consts", bufs=1))
        ztp = ctx.enter_context(tc.tile_pool(name="ztp", bufs=1))
        psum = ctx.enter_context(tc.tile_pool(name="psum", bufs=1, space="PSUM"))
        distp = ctx.enter_context(tc.tile_pool(name="distp", bufs=1))
        ep = ctx.enter_context(tc.tile_pool(name="ep", bufs=1))
        outp = ctx.enter_context(tc.tile_pool(name="outp", bufs=1))

        # ---- input DMAs on the SP HWDGE queue ----
        sb_zt = []
        for v in (0, 1):
            t_ = ztp.tile([D, W], bf16, tag=f"zt{v}", name=f"sb_zt{v}")
            sb_zt.append(t_)
        sb_c = consts.tile([128, 256], bf16, tag="consts", name="sb_c")
        nc.sync.dma_start(sb_zt[0][:, 0:1024], zt_d[0][:, 0:1024])
        nc.sync.dma_start(sb_c[:], consts_d)
        nc.sync.dma_start(sb_zt[0][:, 1024:1280], zt_d[0][:, 1024:1280])
        nc.sync.dma_start(sb_zt[1][:], zt_d[1])
        ident = sb_c[:, 0:128]
        ibig = sb_c[:, 128:256]

        ones = consts.tile([128, 128], bf16, tag="ones", name="ones")
        nc.vector.memset(ones[:], 1.0)
        negh = consts.tile([128, 128], bf16, tag="negh", name="negh")
        nc.vector.memset(negh[:], -0.5)

        # PSUM layout: 3 rotating 2-bank slabs + 1-bank extra + 1-bank outP.
        # outP doubles as the output staging area (the out DMA reads PSUM
        # directly; no SBUF export tile):
        #   0..27 colsums (7 per slab), 28/29 align, 30..33 main rowsums
        #   (33 = ACT accum of the last Exp), 34..37 extra rowsums,
        #   38..41 column norms (internal: bias source).
        outP = psum.tile([128, 42], f32, tag="out", name="outP")
        pex = psum.tile([128, 4, 128], f32, tag="pex", name="pex")

        # PE pstate warm-up: harmless matmuls into pex (overwritten later)
        for _ in range(N_WARMUP):
            nc.tensor.matmul(pex[:, 0, :], ones[:], ones[:],
                             start=True, stop=True)

        # dummy early Sqrt on a const: the sqrt-table load attaches to this
        # instruction's (trivial) waits and runs in the idle head instead of
        # gating the first real Sqrt.
        dummy = outp.tile([128, 1], f32, tag="dummy", name="dummy")
        nc.scalar.activation(dummy[:], ones[:, 0:1], AF.Sqrt,
                             bias=0.0, scale=1.0)

        # ---- sq = zt*zt (DVE); n_i colnorms for the f32 Sqrt bias ----
        sq = []
        for v in (0, 1):
            s_ = outp.tile([128, W], bf16, tag=f"sq{v}", name=f"sq{v}")
            sq.append(s_)
        nc.vector.tensor_mul(sq[0][:, 0:1024], sb_zt[0][:, 0:1024],
                             sb_zt[0][:, 0:1024])
        nc.vector.tensor_mul(sq[0][:, 1024:1280], sb_zt[0][:, 1024:1280],
                             sb_zt[0][:, 1024:1280])
        nc.vector.tensor_mul(sq[1][:], sb_zt[1][:], sb_zt[1][:])

        nrow = outp.tile([128, 4], f32, tag="nrow", name="nrow")
        for v in (0, 1):
            for t in range(2):
                nc.tensor.matmul(outP[:, 38 + 2 * v + t:39 + 2 * v + t],
                                 sq[v][:, t * 128:(t + 1) * 128], ones[:, 0:1],
                                 start=True, stop=True)
            nc.vector.tensor_copy(nrow[:, 2 * v:2 * v + 2],
                                  outP[:, 38 + 2 * v:40 + 2 * v])

        # ---- main slabs: ring distance 0..7 per (view, chunk) ----
        dists = {}
        for v in (0, 1):
            for t in range(2):
                P = psum.tile([128, MAIN], f32, tag="slab", bufs=3, name="P")
                w0 = t * 128
                lhsT = sb_zt[v][:, t * 128:(t + 1) * 128]
                for s in range(2):
                    sl = slice(s * 512, (s + 1) * 512)
                    wsl = slice(w0 + s * 512, w0 + (s + 1) * 512)
                    nc.tensor.matmul(P[:, sl], lhsT, sb_zt[v][:, wsl],
                                     start=True, stop=False)
                    nc.tensor.matmul(P[:, sl], negh[:], sq[v][:, wsl],
                                     start=False, stop=(s != 0))
                nc.tensor.matmul(P[:, 0:128], ident, ibig,
                                 start=False, stop=True)
                idx = v * 2 + t
                dist = distp.tile([128, MAIN], f32, tag=f"dist{idx}",
                                  name=f"dist{idx}")
                nc.scalar.activation(dist[:], P[:], AF.Sqrt,
                                     bias=nrow[:, idx:idx + 1], scale=-2.0)
                dists[idx] = dist

        # ---- extra slab: the four distance-8 blocks, one per (view,chunk) ----
        for q in range(4):
            v, t = q // 2, q % 2
            csl = slice(1024 + t * 128, 1024 + (t + 1) * 128)
            lhsT = sb_zt[v][:, t * 128:(t + 1) * 128]
            nc.tensor.matmul(pex[:, q, :], lhsT, sb_zt[v][:, csl],
                             start=True, stop=False)
            nc.tensor.matmul(pex[:, q, :], negh[:], sq[v][:, csl],
                             start=False, stop=False)
            nc.tensor.matmul(pex[:, q, :], sq[v][:, t * 128:(t + 1) * 128],
                             negh[:, 0:128], start=False, stop=True)
        dist_ex = distp.tile([128, 4, 128], f32, tag="dist_ex", name="dist_ex")
        nc.scalar.activation(dist_ex[:], pex[:], AF.Sqrt, bias=0.0, scale=-2.0)

        # ---- align term: ||z0_i - z1_i||^2 for this core's 256 rows ----
        adiff = outp.tile([128, 256], bf16, tag="adiff", name="adiff")
        nc.vector.tensor_sub(adiff[:], sb_zt[0][:, 0:256], sb_zt[1][:, 0:256])
        asq = outp.tile([128, 256], bf16, tag="asq", name="asq")
        nc.vector.tensor_mul(asq[:], adiff[:], adiff[:])
        for h in range(2):
            nc.tensor.matmul(outP[:, 28 + h:29 + h],
                             asq[:, h * 128:(h + 1) * 128], ones[:, 0:1],
                             start=True, stop=True)

        # ---- serialize all Exp after every Sqrt: Exp's scale operand is
        # derived from all 5 dist tiles -> exactly 2 ACT table loads.
        neg = outp.tile([128, 1], f32, tag="neg", name="neg")
        nc.vector.tensor_scalar(neg[:], dists[0][:, 0:1], 0.0, -1.0,
                                mb.AluOpType.mult, mb.AluOpType.add)
        for dep in (dists[1], dists[2], dists[3]):
            nc.vector.tensor_tensor(neg[:], neg[:], dep[:, 0:1],
                                    mb.AluOpType.bypass)
        # (no dist_ex link: by the time dist3 exists, Sx's inputs are long
        # ready, so the scheduler's priority order keeps Sx before any Exp)

        # ---- Exp passes + row sums + column sums (stationary matmuls) ----
        # e_ex first (its DVE reduce then overlaps the main Exp chain);
        # the LAST Exp uses ACT accum_out so no DVE reduce lands in the tail.
        # Main-slab row sums use tensor_scalar+accum_out (bypass op, dummy
        # out) which gets the 2x bf16 DVE mode that tensor_reduce lacks.
        export = outp.tile([128, 38], f32, tag="export", name="export")
        e_ex = ep.tile([128, 4, 128], bf16, tag="e_ex", name="e_ex")
        nc.scalar.activation(e_ex[:], dist_ex[:], AF.Exp, scale=neg[:, 0:1])
        nc.vector.tensor_reduce(export[:, 34:38], e_ex[:],
                                mb.AxisListType.X, mb.AluOpType.add)
        rdump = ep.tile([128, MAIN], bf16, tag="rdump", name="rdump")
        for idx in range(4):
            E = ep.tile([128, MAIN], bf16, tag=f"e{idx}", name=f"e{idx}")
            if idx == 3:
                nc.scalar.activation(E[:], dists[idx][:], AF.Exp,
                                     scale=neg[:, 0:1],
                                     accum_out=export[:, 33:34])
            else:
                nc.scalar.activation(E[:], dists[idx][:], AF.Exp,
                                     scale=neg[:, 0:1])
                nc.vector.tensor_scalar(rdump[:], E[:], 1.0, 0.0,
                                        mb.AluOpType.mult, mb.AluOpType.add,
                                        accum_out=export[:, 30 + idx:31 + idx])
            for b in range(1, 8):
                col = idx * 7 + (b - 1)
                nc.tensor.matmul(outP[:, col:col + 1],
                                 E[:, b * 128:(b + 1) * 128], ones[:, 0:1],
                                 start=True, stop=True)

        # PSUM->SBUF staging on ACT (Copy is in every table set: no reload),
        # right after the last Exp's accumulator read.
        nc.scalar.copy(export[:, 0:30], outP[:, 0:30])
        nc.sync.dma_start(out_d, export[:])

    nc.compile()
    return nc


def _prep_inputs(z0: np.ndarray, z1: np.ndarray):
    """Per-core input maps: rotate columns so core c's rows come first."""
    bf = ml_dtypes.bfloat16
    zs = [np.ascontiguousarray(z0, np.float32), np.ascontiguousarray(z1, np.float32)]
    eye = np.eye(128, dtype=np.float32)
    consts = np.concatenate([eye, -BIG * eye], axis=1).astype(bf)  # [128, 256]
    in_maps = []
    for c in range(N_CORES):
        order = (np.arange(W) + c * R) % B
        m = {"consts": consts}
        for v in (0, 1):
            zr = zs[v][order]                                    # [W, D] rotated
            m[f"zt{v}"] = np.ascontiguousarray(zr.T).astype(bf)  # [D, W]
        in_maps.append(m)
    return in_maps


def kernel(z0: np.ndarray, z1: np.ndarray) -> np.ndarray:
    from concourse.bass_utils import run_bass_kernel_spmd

    if "nc" not in _cache:
        _cache["nc"] = _build()
    nc = _cache["nc"]

    in_maps = _prep_inputs(z0, z1)
    res = run_bass_kernel_spmd(nc, in_maps, core_ids=list(range(N_CORES)))

    rowsums = np.zeros((2, B), np.float64)   # [view, global row]
    alignsq = np.empty((B,), np.float64)
    for c in range(N_CORES):
        out = res.results[c]["out"].astype(np.float64)   # [128, 38]
        for v in (0, 1):
            for t in range(2):
                idx = v * 2 + t
                own = ((2 * c + t) % 16) * 128
                # main (distances 0..7) + extra (distance 8) row sums
                rowsums[v, own:own + 128] += out[:, 30 + idx] + out[:, 34 + idx]
                # received column sums (distances 1..7, transposed rows)
                for b in range(1, 8):
                    g = ((2 * c + t + b) % 16) * 128
                    rowsums[v, g:g + 128] += out[:, idx * 7 + (b - 1)]
        alignsq[c * R:c * R + 128] = out[:, 28]
        alignsq[c * R + 128:c * R + 256] = out[:, 29]

    align_loss = np.sqrt(alignsq).mean()
    lme = np.log(rowsums) - LOG_NM1             # [2, B]
    entropy_loss = lme.mean()
    return np.float32(align_loss - entropy_loss)
